# revision 31
# baseline (speedup 1.0000x reference)
"""Trainium2 Bass kernel for single-token (decode) multi-head attention.

Problem: q [8,32,1,128], k/v [8,32,4096,128], mask [8,1,1,4096] (fp32)
  out = softmax(q*scale @ k^T + mask) @ v          -> [8,32,1,128]

Sharding: batch across the 8 NeuronCores (B=8 -> 1 batch per core, all 32
heads on-core; no cross-core communication).

The kernel is HBM-bandwidth bound (must stream all of K and V once).
K/V/q are cast to fp16 on the host (rel err ~4e-4): 64MB/core @ ~358GB/s
=> ~180us DMA roofline. The compute structure keeps every engine well
under that budget:

  - K is staged transposed (host): kt [H=128 partitions, KV free].
  - scores mul: ONE DVE tensor_scalar_mul per head (q as per-partition
    [P,1] scalar) -> prod[h, kv] = k*q. 4x DVE mode, ~1.1us/head.
  - H-reduction on PE: stationary E_n = [128, 32] with column n all-ones
    => psum[32, 512] chunk accumulates row n = head n's scores. All 32
    heads stack into [32 partitions, 4096] scores. Mask is pre-added via
    a 1-partition ones matmul per chunk.
  - softmax: 8 ACT exps [32, 512] (all heads at once) + accum partials;
    row-sum via ACT accum; 1 DVE reciprocal; weights normalized BEFORE
    AV by one DVE tensor_scalar_mul [32, 4096].
  - weights transposed back to kv-partition layout via 32 PE transposes
    [32,128] -> [128,32] (+ ACT copies to SBUF).
  - AV: per (head, chunk): psum[1,128] += peT[:, (c,n)].T @ v[:, chunk],
    v staged (host) as [128 kv-in-chunk partitions, (chunk, h)].
  - V DMAs ride the gpsimd (SWDGE) queue so deep prefetch can stall
    without blocking ACT-engine compute (deadlock avoidance).
"""

import os

import numpy as np

import concourse.mybir as mybir
import concourse.tile as tile
from concourse import bacc
from concourse.bass_utils import run_bass_kernel_spmd

B, N, T, H, KV = 8, 32, 1, 128, 4096
SCALE = float(H) ** -0.5
P = 128          # partitions
C = KV // P      # 32 kv chunks of 128
NCH = 8          # psum score chunks (512 wide)
CW = KV // NCH   # 512
F32 = mybir.dt.float32
F16 = mybir.dt.float16

V_BUFS = 13

_NC_CACHE = None
LAST_RESULT = None  # BassKernelResults of the most recent run (for test harness)


def _build(n_heads=N):
    nc = bacc.Bacc()
    kt_d = nc.dram_tensor("kt", [N, H, KV], F16, kind="ExternalInput")
    v_d = nc.dram_tensor("vt", [N, P, C * H], F16, kind="ExternalInput")
    qt_d = nc.dram_tensor("qt", [H, N], F32, kind="ExternalInput")
    m_d = nc.dram_tensor("m1p", [1, KV], F16, kind="ExternalInput")
    en_d = nc.dram_tensor("en", [P, N * N], F16, kind="ExternalInput")
    id_d = nc.dram_tensor("id32", [32, 32], F16, kind="ExternalInput")
    o_d = nc.dram_tensor("out", [1, N * H], F32, kind="ExternalOutput")

    with tile.TileContext(nc) as tc:
        with (
            tc.tile_pool(name="const", bufs=1) as const,
            tc.tile_pool(name="kp", bufs=4) as kp,
            tc.tile_pool(name="vp", bufs=V_BUFS) as vp,
            tc.tile_pool(name="prod", bufs=4) as prp,
            tc.tile_pool(name="spsum", bufs=1, space="PSUM") as spp,
            tc.tile_pool(name="tpsum", bufs=2, space="PSUM") as tpp,
            tc.tile_pool(name="po", bufs=3, space="PSUM") as pop,
            tc.tile_pool(name="srp", bufs=1, space="PSUM") as srp,
        ):
            # ---- constants (scalar HWDGE queue, before anything else) ----
            qt = const.tile([H, N], F32)
            nc.scalar.dma_start(out=qt[:], in_=qt_d[:])
            m1p = const.tile([1, KV], F16)
            nc.scalar.dma_start(out=m1p[:], in_=m_d[:])
            en = const.tile([P, N * N], F16)
            nc.scalar.dma_start(out=en[:], in_=en_d[:])
            id32 = const.tile([32, 32], F16)
            nc.scalar.dma_start(out=id32[:], in_=id_d[:])
            ones_row = const.tile([1, N], F16)
            nc.vector.memset(ones_row[:], 1.0)

            p_e = const.tile([32, KV], F16)      # exp(scores), stacked heads
            s_part = const.tile([32, NCH], F32)  # per-chunk partial sums
            s_scr = const.tile([32, NCH], F32)   # scratch for rowsum copy
            s_sum = const.tile([32, 1], F32)
            s16 = const.tile([32, 1], F16)
            # 1/rowsum as a row, replicated at partitions {0,32,64,96}
            recip4 = const.tile([97, N], F32)
            peT = const.tile([P, C * 32], F16)   # transposed weights (c, n)
            # 4 output rows at partitions {0,32,64,96}: row j holds heads
            # n = 4m+j (col-strip j's AV results land on partition 32j).
            out_rows = const.tile([97, (N // 4) * H], F32)

            # ---- phase A: scores for all heads, stacked in PSUM ----
            # 8 score chunks [32, 512], packed 4-per-bank at partition
            # offsets {0,32,64,96} (explicit tile_position col strips).
            sbank = [spp.tile([P, CW], F32, name=f"sbank{i}") for i in range(2)]

            def s_chunk(c):
                off = 32 * (c % 4)
                return sbank[c // 4][off:off + 32, :], (0, off)

            for c in range(NCH):
                ap, tpos = s_chunk(c)
                nc.tensor.matmul(
                    ap,
                    lhsT=ones_row[:],
                    rhs=m1p[0:1, c * CW:(c + 1) * CW],
                    start=True, stop=False,
                    skip_group_check=True,
                    tile_position=tpos,
                )

            for n in range(n_heads):
                kt_sb = kp.tile([H, KV], F16)
                nc.sync.dma_start(out=kt_sb[:], in_=kt_d[n])

                prod = prp.tile([H, KV], F16)
                nc.vector.tensor_scalar_mul(
                    out=prod[:], in0=kt_sb[:], scalar1=qt[:, n:n + 1],
                )
                for c in range(NCH):
                    ap, tpos = s_chunk(c)
                    nc.tensor.matmul(
                        ap,
                        lhsT=en[:, n * N:(n + 1) * N],
                        rhs=prod[:, c * CW:(c + 1) * CW],
                        start=False, stop=(n == n_heads - 1),
                        skip_group_check=True,
                        tile_position=tpos,
                    )

            # V rides the same sync queue AFTER all K: FIFO gives K strict
            # priority (phase A is K-gated), then V streams back-to-back
            # while AV consumes tiles at matching pace.
            v_tiles = []
            for n in range(n_heads):
                v_sb = vp.tile([P, C * H], F16)
                nc.sync.dma_start(out=v_sb[:], in_=v_d[n])
                v_tiles.append(v_sb)

            # ---- softmax (all heads at once) ----
            # exp(c) immediately enables the 4 weight-transposes of its
            # 512-wide span (normalization is deferred to the output).
            for c in range(NCH):
                ap, _ = s_chunk(c)
                nc.scalar.activation(
                    out=p_e[:, c * CW:(c + 1) * CW],
                    in_=ap,
                    func=mybir.ActivationFunctionType.Exp,
                    accum_out=s_part[:, c:c + 1],
                )
                for cc in range(4 * c, 4 * c + 4):
                    pt = tpp.tile([P, 32], F16, name="pt")
                    nc.tensor.transpose(
                        pt[:], p_e[:, cc * P:(cc + 1) * P], id32[:],
                    )
                    nc.scalar.activation(
                        out=peT[:, cc * 32:(cc + 1) * 32], in_=pt[:],
                        func=mybir.ActivationFunctionType.Copy,
                    )

            # row-sums -> [1, 32] reciprocal row (via tiny PE transpose)
            nc.scalar.activation(
                out=s_scr[:], in_=s_part[:],
                func=mybir.ActivationFunctionType.Copy,
                accum_out=s_sum[:],
            )
            nc.vector.tensor_copy(s16[:], s_sum[:])
            srow = srp.tile([97, N], F32)
            for j in range(4):
                nc.tensor.matmul(
                    srow[32 * j:32 * j + 1, :], lhsT=s16[:], rhs=id32[:],
                    start=True, stop=True,
                    tile_position=(0, 32 * j), skip_group_check=True,
                )
                nc.vector.reciprocal(
                    out=recip4[32 * j:32 * j + 1, :],
                    in_=srow[32 * j:32 * j + 1, :],
                )

            # ---- AV + output ----
            # Quads of 4 heads run concurrently on the 4 PE col-strips
            # (tile_position (0, 32j)); head 4m+j accumulates in psum
            # partition 32j.
            for m in range(n_heads // 4):
                pob = pop.tile([97, H], F32, name="pob")
                for c in range(C):
                    for j in range(4):
                        n = 4 * m + j
                        nc.tensor.matmul(
                            pob[32 * j:32 * j + 1, :],
                            lhsT=peT[:, c * 32 + n:c * 32 + n + 1],
                            rhs=v_tiles[n][:, c * H:(c + 1) * H],
                            start=(c == 0), stop=(c == C - 1),
                            tile_position=(0, 32 * j),
                            skip_group_check=True,
                        )
                for j in range(4):
                    n = 4 * m + j
                    nc.scalar.activation(
                        out=out_rows[32 * j:32 * j + 1, m * H:(m + 1) * H],
                        in_=pob[32 * j:32 * j + 1, :],
                        func=mybir.ActivationFunctionType.Copy,
                        scale=recip4[32 * j:32 * j + 1, n:n + 1],
                    )
                # ship this quad's 4 head-outputs immediately (one DMA
                # over partitions {0,32,64,96} -> 4 contiguous HBM rows).
                # On the scalar queue: it must NOT sit between v-dispatches
                # on the sync queue, or it stalls the V stream.
                nc.scalar.dma_start(
                    out=o_d[0:1, 4 * m * H:(4 * m + 4) * H],
                    in_=out_rows[0:97:32, m * H:(m + 1) * H],
                )
    nc.finalize()
    return nc


_EN_CONST = None
_ID_CONST = None


def _consts():
    global _EN_CONST, _ID_CONST
    if _EN_CONST is None:
        en = np.zeros((P, N, N), dtype=np.float16)
        for n in range(N):
            en[:, n, n] = 1.0
        _EN_CONST = np.ascontiguousarray(en.reshape(P, N * N))
        _ID_CONST = np.ascontiguousarray(np.eye(32, dtype=np.float16))
    return _EN_CONST, _ID_CONST


def kernel(q, k, v, mask):
    global _NC_CACHE, LAST_RESULT
    q = np.asarray(q, dtype=np.float32)
    k16 = np.asarray(k, dtype=np.float16)
    v16 = np.asarray(v, dtype=np.float16)
    mask16 = np.asarray(mask, dtype=np.float16)

    if _NC_CACHE is None:
        _NC_CACHE = _build()
    nc = _NC_CACHE
    en, id32 = _consts()

    in_maps = []
    for b in range(B):
        # kt: [N, H, KV]
        kt = np.ascontiguousarray(k16[b].transpose(0, 2, 1))
        # v chunk-partition layout: vt[n, p, c*H + h] = v[n, c*128 + p, h]
        vt = np.ascontiguousarray(
            v16[b].reshape(N, C, P, H).transpose(0, 2, 1, 3).reshape(N, P, C * H)
        )
        qt = np.ascontiguousarray((q[b, :, 0, :] * SCALE).T.astype(np.float32))
        in_maps.append({
            "kt": kt,
            "vt": vt,
            "qt": qt,
            "m1p": np.ascontiguousarray(mask16[b, 0, 0, :].reshape(1, KV)),
            "en": en,
            "id32": id32,
        })

    res = run_bass_kernel_spmd(
        nc,
        in_maps,
        core_ids=list(range(B)),
        trace=bool(int(os.environ.get("KERNEL_TRACE", "0"))),
    )
    LAST_RESULT = res
    out = np.stack([r["out"].reshape(N, H) for r in res.results])
    return out[:, :, None, :].astype(np.float32)


# revision 32
# speedup vs baseline: 1.0580x; 1.0580x over previous
"""Trainium2 Bass kernel for single-token (decode) multi-head attention.

Problem: q [8,32,1,128], k/v [8,32,4096,128], mask [8,1,1,4096] (fp32)
  out = softmax(q*scale @ k^T + mask) @ v          -> [8,32,1,128]

Sharding: batch across the 8 NeuronCores (B=8 -> 1 batch per core, all 32
heads on-core; no cross-core communication).

The kernel is HBM-bandwidth bound (must stream all of K and V once).
K/V/q are cast to fp16 on the host (rel err ~4e-4): 64MB/core @ ~358GB/s
=> ~180us DMA roofline. The compute structure keeps every engine well
under that budget:

  - K is staged transposed (host): kt [H=128 partitions, KV free].
  - scores mul: ONE DVE tensor_scalar_mul per head (q as per-partition
    [P,1] scalar) -> prod[h, kv] = k*q. 4x DVE mode, ~1.1us/head.
  - H-reduction on PE: stationary E_n = [128, 32] with column n all-ones
    => psum[32, 512] chunk accumulates row n = head n's scores. All 32
    heads stack into [32 partitions, 4096] scores. Mask is pre-added via
    a 1-partition ones matmul per chunk.
  - softmax: 8 ACT exps [32, 512] (all heads at once) + accum partials;
    row-sum via ACT accum; 1 DVE reciprocal; weights normalized BEFORE
    AV by one DVE tensor_scalar_mul [32, 4096].
  - weights transposed back to kv-partition layout via 32 PE transposes
    [32,128] -> [128,32] (+ ACT copies to SBUF).
  - AV: per (head, chunk): psum[1,128] += peT[:, (c,n)].T @ v[:, chunk],
    v staged (host) as [128 kv-in-chunk partitions, (chunk, h)].
  - V DMAs ride the gpsimd (SWDGE) queue so deep prefetch can stall
    without blocking ACT-engine compute (deadlock avoidance).
"""

import os

import numpy as np

import concourse.mybir as mybir
import concourse.tile as tile
from concourse import bacc
from concourse.bass_utils import run_bass_kernel_spmd

B, N, T, H, KV = 8, 32, 1, 128, 4096
SCALE = float(H) ** -0.5
P = 128          # partitions
C = KV // P      # 32 kv chunks of 128
NCH = 8          # psum score chunks (512 wide)
CW = KV // NCH   # 512
F32 = mybir.dt.float32
F16 = mybir.dt.float16
F8 = mybir.dt.float8e4

V_BUFS = 16

_NC_CACHE = None
LAST_RESULT = None  # BassKernelResults of the most recent run (for test harness)


def _build(n_heads=N):
    nc = bacc.Bacc()
    kt_d = nc.dram_tensor("kt", [N, H, KV], F16, kind="ExternalInput")
    v_d = nc.dram_tensor("vt", [N, P, C * H], F16, kind="ExternalInput")
    qt_d = nc.dram_tensor("qt", [H, N], F32, kind="ExternalInput")
    m_d = nc.dram_tensor("m1p", [1, KV], F8, kind="ExternalInput")
    en_d = nc.dram_tensor("en", [P, N * N], F16, kind="ExternalInput")
    id_d = nc.dram_tensor("id32", [32, 32], F16, kind="ExternalInput")
    o_d = nc.dram_tensor("out", [1, N * H], F32, kind="ExternalOutput")

    with tile.TileContext(nc) as tc:
        with (
            tc.tile_pool(name="const", bufs=1) as const,
            tc.tile_pool(name="kp", bufs=3) as kp,
            tc.tile_pool(name="vp", bufs=V_BUFS) as vp,
            tc.tile_pool(name="prod", bufs=3) as prp,
            tc.tile_pool(name="spsum", bufs=1, space="PSUM") as spp,
            tc.tile_pool(name="tpsum", bufs=2, space="PSUM") as tpp,
            tc.tile_pool(name="po", bufs=3, space="PSUM") as pop,
            tc.tile_pool(name="srp", bufs=1, space="PSUM") as srp,
        ):
            # ---- constants (scalar HWDGE queue, before anything else) ----
            qt = const.tile([H, N], F32)
            nc.scalar.dma_start(out=qt[:], in_=qt_d[:])
            m1p = const.tile([1, KV], F8)
            nc.scalar.dma_start(out=m1p[:], in_=m_d[:])
            en = const.tile([P, N * N], F16)
            nc.scalar.dma_start(out=en[:], in_=en_d[:])
            id32 = const.tile([32, 32], F16)
            nc.scalar.dma_start(out=id32[:], in_=id_d[:])
            ones_row = const.tile([1, N], F16)
            nc.vector.memset(ones_row[:], 1.0)

            p_e = const.tile([32, KV], F16)      # exp(scores), stacked heads
            s_part = const.tile([32, NCH], F32)  # per-chunk partial sums
            s_scr = const.tile([32, NCH], F32)   # scratch for rowsum copy
            s_sum = const.tile([32, 1], F32)
            s16 = const.tile([32, 1], F16)
            # 1/rowsum as a row, replicated at partitions {0,32,64,96}
            recip4 = const.tile([97, N], F32)
            peT = const.tile([P, C * 32], F16)   # transposed weights (c, n)
            # 4 output rows at partitions {0,32,64,96}: row j holds heads
            # n = 4m+j (col-strip j's AV results land on partition 32j).
            out_rows = const.tile([97, (N // 4) * H], F32)

            # ---- phase A: scores for all heads, stacked in PSUM ----
            # 8 score chunks [32, 512], packed 4-per-bank at partition
            # offsets {0,32,64,96} (explicit tile_position col strips).
            sbank = [spp.tile([P, CW], F32, name=f"sbank{i}") for i in range(2)]

            def s_chunk(c):
                off = 32 * (c % 4)
                return sbank[c // 4][off:off + 32, :], (0, off)

            for c in range(NCH):
                ap, tpos = s_chunk(c)
                nc.tensor.matmul(
                    ap,
                    lhsT=ones_row[:],
                    rhs=m1p[0:1, c * CW:(c + 1) * CW],
                    start=True, stop=False,
                    skip_group_check=True,
                    tile_position=tpos,
                )

            for n in range(n_heads):
                kt_sb = kp.tile([H, KV], F16)
                nc.sync.dma_start(out=kt_sb[:], in_=kt_d[n])

                prod = prp.tile([H, KV], F16)
                nc.vector.tensor_scalar_mul(
                    out=prod[:], in0=kt_sb[:], scalar1=qt[:, n:n + 1],
                )
                for c in range(NCH):
                    ap, tpos = s_chunk(c)
                    nc.tensor.matmul(
                        ap,
                        lhsT=en[:, n * N:(n + 1) * N],
                        rhs=prod[:, c * CW:(c + 1) * CW],
                        start=False, stop=(n == n_heads - 1),
                        skip_group_check=True,
                        tile_position=tpos,
                    )

            # V rides the same sync queue AFTER all K: FIFO gives K strict
            # priority (phase A is K-gated), then V streams back-to-back
            # while AV consumes tiles at matching pace.
            v_tiles = []
            for n in range(n_heads):
                v_sb = vp.tile([P, C * H], F16)
                nc.sync.dma_start(out=v_sb[:], in_=v_d[n])
                v_tiles.append(v_sb)

            # ---- softmax (all heads at once) ----
            # exp(c) immediately enables the 4 weight-transposes of its
            # 512-wide span (normalization is deferred to the output).
            for c in range(NCH):
                ap, _ = s_chunk(c)
                nc.scalar.activation(
                    out=p_e[:, c * CW:(c + 1) * CW],
                    in_=ap,
                    func=mybir.ActivationFunctionType.Exp,
                    accum_out=s_part[:, c:c + 1],
                )
                for cc in range(4 * c, 4 * c + 4):
                    pt = tpp.tile([P, 32], F16, name="pt")
                    nc.tensor.transpose(
                        pt[:], p_e[:, cc * P:(cc + 1) * P], id32[:],
                    )
                    nc.scalar.activation(
                        out=peT[:, cc * 32:(cc + 1) * 32], in_=pt[:],
                        func=mybir.ActivationFunctionType.Copy,
                    )

            # row-sums -> [1, 32] reciprocal row (via tiny PE transpose)
            nc.scalar.activation(
                out=s_scr[:], in_=s_part[:],
                func=mybir.ActivationFunctionType.Copy,
                accum_out=s_sum[:],
            )
            nc.vector.tensor_copy(s16[:], s_sum[:])
            srow = srp.tile([97, N], F32)
            for j in range(4):
                nc.tensor.matmul(
                    srow[32 * j:32 * j + 1, :], lhsT=s16[:], rhs=id32[:],
                    start=True, stop=True,
                    tile_position=(0, 32 * j), skip_group_check=True,
                )
                nc.vector.reciprocal(
                    out=recip4[32 * j:32 * j + 1, :],
                    in_=srow[32 * j:32 * j + 1, :],
                )

            # ---- AV + output ----
            # Quads of 4 heads run concurrently on the 4 PE col-strips
            # (tile_position (0, 32j)); head 4m+j accumulates in psum
            # partition 32j.
            for m in range(n_heads // 4):
                pob = pop.tile([97, H], F32, name="pob")
                for c in range(C):
                    for j in range(4):
                        n = 4 * m + j
                        nc.tensor.matmul(
                            pob[32 * j:32 * j + 1, :],
                            lhsT=peT[:, c * 32 + n:c * 32 + n + 1],
                            rhs=v_tiles[n][:, c * H:(c + 1) * H],
                            start=(c == 0), stop=(c == C - 1),
                            tile_position=(0, 32 * j),
                            skip_group_check=True,
                        )
                for j in range(4):
                    n = 4 * m + j
                    nc.scalar.activation(
                        out=out_rows[32 * j:32 * j + 1, m * H:(m + 1) * H],
                        in_=pob[32 * j:32 * j + 1, :],
                        func=mybir.ActivationFunctionType.Copy,
                        scale=recip4[32 * j:32 * j + 1, n:n + 1],
                    )
                # ship this quad's 4 head-outputs immediately (one DMA
                # over partitions {0,32,64,96} -> 4 contiguous HBM rows).
                # On the scalar queue: it must NOT sit between v-dispatches
                # on the sync queue, or it stalls the V stream.
                nc.scalar.dma_start(
                    out=o_d[0:1, 4 * m * H:(4 * m + 4) * H],
                    in_=out_rows[0:97:32, m * H:(m + 1) * H],
                )
    nc.finalize()
    return nc


_EN_CONST = None
_ID_CONST = None


def _consts():
    global _EN_CONST, _ID_CONST
    if _EN_CONST is None:
        en = np.zeros((P, N, N), dtype=np.float16)
        for n in range(N):
            en[:, n, n] = 1.0
        _EN_CONST = np.ascontiguousarray(en.reshape(P, N * N))
        _ID_CONST = np.ascontiguousarray(np.eye(32, dtype=np.float16))
    return _EN_CONST, _ID_CONST


def kernel(q, k, v, mask):
    global _NC_CACHE, LAST_RESULT
    q = np.asarray(q, dtype=np.float32)
    k16 = np.asarray(k, dtype=np.float16)
    v16 = np.asarray(v, dtype=np.float16)
    import ml_dtypes
    mask8 = np.asarray(mask, dtype=ml_dtypes.float8_e4m3fn)

    if _NC_CACHE is None:
        _NC_CACHE = _build()
    nc = _NC_CACHE
    en, id32 = _consts()

    in_maps = []
    for b in range(B):
        # kt: [N, H, KV]
        kt = np.ascontiguousarray(k16[b].transpose(0, 2, 1))
        # v chunk-partition layout: vt[n, p, c*H + h] = v[n, c*128 + p, h]
        vt = np.ascontiguousarray(
            v16[b].reshape(N, C, P, H).transpose(0, 2, 1, 3).reshape(N, P, C * H)
        )
        qt = np.ascontiguousarray((q[b, :, 0, :] * SCALE).T.astype(np.float32))
        in_maps.append({
            "kt": kt,
            "vt": vt,
            "qt": qt,
            "m1p": np.ascontiguousarray(mask8[b, 0, 0, :].reshape(1, KV)),
            "en": en,
            "id32": id32,
        })

    res = run_bass_kernel_spmd(
        nc,
        in_maps,
        core_ids=list(range(B)),
        trace=bool(int(os.environ.get("KERNEL_TRACE", "0"))),
    )
    LAST_RESULT = res
    out = np.stack([r["out"].reshape(N, H) for r in res.results])
    return out[:, :, None, :].astype(np.float32)
